# revision 38
# baseline (speedup 1.0000x reference)
"""Dilated self-attention TRN2 kernel (nn_DilatedSelfAttention) — v3.

Problem (hardcoded — self-contained):
  x (4, 8192, 128) f32; Wq/Wk/Wv (128,128) f32; indices (14336) i64.
  WS=[2048,4096,8192], RS=[1,2,4], HEAD_IDX=1 -> 7 segments of 2048 per batch:
    seg0..3: windows [2048t, 2048(t+1))   (stride 1)
    seg4:    1 + 2*i, i<2048              (stride 2, odd of [0,4096))
    seg5:    4097 + 2*i                   (stride 2, odd of [4096,8192))
    seg6:    1 + 4*i                      (stride 4, p%4==1)
  Each segment: causal softmax attention; outputs mixed position-wise weighted
  by the reference's max-SHIFTED softmax denominators D~ = sum exp(s - rowmax):
    out[p] = sum_seg (E~ @ V)[p] / sum_seg D~[p],  E~ = exp(s - rowmax).

Sharding (no collectives): core 2b+rho owns batch b, output half rho.
  rho=0 pieces: seg0, seg1, seg4, seg6[q-tiles 0..7]
  rho=1 pieces: seg2, seg3, seg5, seg6[q-tiles 8..15]
Every query's output position lands in the core's own half, and all segments
touching a position are on the same core, so the scatter-add is local.

v4.1 changes vs v2:
  - Per-slot flow unchanged (exp -> DVE max -> recip -> in-place scale with
    row-sum accum D~ -> DMA-xbar transpose): the scale is per-PARTITION (q),
    the only legal/cheap place for the e^{-m} shift (compute engines require
    identical start partitions on all operands, so a free-dim rec row would
    need a flatten DMA whose latency stalls the EV/PSUM recycle chain).
  - den path per 4-slot GROUP (was per piece): PE transpose [128,4]->[4,128]
    -> DVE copy -> SWDGE flatten to a partition-0 [1,512] row -> gpsimd
    strided scatter into DenT. Off the critical path; enables the early
    epilogue below.
  - Epilogue v4: per 512-col out quarter r: DVE recip -> Pool
    partition_broadcast -> DVE f16 mult -> DMA-xbar transpose [128,4,128]
    -> DVE f32 upcast -> SP/ACT HWDGE out-DMA (4D dst AP). No fp32 PE
    rank-1, no PE transposes. Quarters r=7..4 are emitted interleaved with
    piece3's last EV group (their Num/Den deps complete early); r=3..0 after.
    NOTE: a SWDGE out-DMA with f16->f32 cast compiles but the NEFF then
    fails to LOAD (LoadExecutable error) — hence the DVE upcast + HWDGE.
  - Et transposes alternate between the two physical HWDGE rings
    (t%2 ? nc.sync : nc.scalar) so descriptor generation overlaps.
  - Tail events are emitted in slot order (see the comment in at()): out-of-
    order tail emission lets the evqp pool hand ev(3,0) a PSUM buffer before
    ev(3,1)'s scatter is emitted — a deterministic clobber.
"""
import math
import os
import sys

sys.path.insert(0, "/opt/trn_rl_repo")

import numpy as np

import concourse.bass as bass
import concourse.bacc as bacc
import concourse.mybir as mybir
import concourse.tile as tile
from concourse.bass_utils import run_bass_kernel_spmd
from concourse.masks import make_identity

f32 = mybir.dt.float32
f16 = mybir.dt.float16

B, N, C = 4, 8192, 128
S = 2048
HALF = N // 2
NPIECE = 4
NEG = -30000.0

NS_P = [16, 16, 16, 8]       # query slots per piece
KOFF_P = [0, 0, 0, 8]        # key-chunk offset (piece3 queries start at tile 8)
NCH_P = [16, 16, 16, 16]     # key chunks held
BASE_P = [0, 2048, 1, 1]     # local position base per piece
STRIDE_P = [1, 1, 2, 4]      # local position stride per piece
NG_P = [4, 4, 4, 2]          # 4-slot groups per piece


def build_nc(loop_k=None, skip_rs=False, skip=(), unroll_k=None, qpoff=4, lead3=4, etbufs=6):
    nc = bacc.Bacc(None, target_bir_lowering=False)

    bxT4 = nc.dram_tensor("bxT4", [NPIECE, C, S], f16, kind="ExternalInput")
    qx3 = nc.dram_tensor("qx3", [C, 1024], f16, kind="ExternalInput")
    maskD = nc.dram_tensor("maskD", [128, 128], f16, kind="ExternalInput")
    maskW = nc.dram_tensor("maskW", [128, 1152], f16, kind="ExternalInput")
    Mt = nc.dram_tensor("Mt", [C, C], f16, kind="ExternalInput")
    Wvt = nc.dram_tensor("Wvt", [C, C], f16, kind="ExternalInput")
    out_half = nc.dram_tensor("out_half", [HALF, C], f32, kind="ExternalOutput")

    with tile.TileContext(nc) as tc:
        with (
            tc.tile_pool(name="fix", bufs=1) as fix,
            tc.tile_pool(name="bx", bufs=3) as bxp,
            tc.tile_pool(name="qpt", bufs=2) as qptp,
            tc.tile_pool(name="vsl", bufs=2) as vslp,
            tc.tile_pool(name="Et", bufs=etbufs) as Etp,
            tc.tile_pool(name="ETG", bufs=4) as etgp,
            tc.tile_pool(name="slab", bufs=2) as slabp,
            tc.tile_pool(name="small", bufs=2) as smp,
            tc.tile_pool(name="epi", bufs=3) as epi,
            tc.tile_pool(name="spool", bufs=3, space="PSUM") as spool,
            tc.tile_pool(name="evq", bufs=2, space="PSUM") as evqp,
        ):
            # ---- fixed tensors ----
            ident = fix.tile([128, 128], f32)
            make_identity(nc, ident[:])
            ident16 = fix.tile([128, 128], f16)
            nc.gpsimd.tensor_copy(ident16[:], ident[:])

            m16 = fix.tile([C, C], f16)
            wv16 = fix.tile([C, C], f16)
            mD = fix.tile([128, 128], f16)
            mW = fix.tile([128, 1152], f16)
            qx16 = fix.tile([C, 1024], f16)
            nc.sync.dma_start(m16[:], Mt[:])
            nc.sync.dma_start(wv16[:], Wvt[:])
            nc.sync.dma_start(mD[:], maskD[:])
            nc.sync.dma_start(mW[:], maskW[:])
            nc.sync.dma_start(qx16[:], qx3[:])

            NumT = fix.tile([C, HALF], f32)
            DenT = fix.tile([1, HALF], f32)

            def _one_iter(su):
                st_bx = [None] * NPIECE
                st_qpt = [None] * NPIECE
                st_vsl = [None] * NPIECE
                st_du = [None] * NPIECE
                st_max = [None] * NPIECE
                st_rec = [None] * NPIECE
                st_etg = {}

                def emit_dma(p):
                    # gpsimd-dispatched: keeps the SP DMA queue free for the
                    # latency-critical Et transposes
                    bx16 = bxp.tile([C, S], f16, name=f"bx{p}{su}", tag="bx")
                    nc.gpsimd.dma_start(bx16[:], bxT4[p])
                    st_bx[p] = bx16

                def emit_qp_block(p, h):
                    bx16 = st_bx[p]
                    qsrc = bx16 if p < 3 else qx16
                    if h == 0:
                        st_qpt[p] = qptp.tile([C, S], f16, name=f"qpt{p}{su}", tag="qpt")
                        st_du[p] = slabp.tile([128, 16], f32, name=f"du{p}{su}", tag="du")
                        st_max[p] = slabp.tile([128, 16], f32, name=f"max{p}{su}", tag="max")
                        st_rec[p] = slabp.tile([128, 16], f32, name=f"rec{p}{su}", tag="rec")
                    qps = spool.tile([128, 1024], f32, tag="s", name=f"qps{p}{h}{su}")
                    for i in range(2):
                        nc.tensor.matmul(
                            qps[:, 512 * i : 512 * i + 512],
                            m16[:],
                            qsrc[:, 1024 * h + 512 * i : 1024 * h + 512 * i + 512],
                            start=True, stop=True, skip_group_check=True,
                        )
                    nc.vector.tensor_copy(st_qpt[p][:, 1024 * h : 1024 * h + 1024], qps[:])

                def emit_v_half(p, half):
                    bx16 = st_bx[p]
                    if half == 0:
                        st_vsl[p] = vslp.tile([128, S], f16, name=f"vsl{p}{su}", tag="vsl")
                    vps = spool.tile([128, 1024], f32, tag="s", name=f"vps{p}{half}{su}")
                    for q in range(8):
                        cch = 8 * half + q
                        nc.tensor.matmul(
                            vps[:, 128 * q : 128 * q + 128],
                            bx16[:, 128 * cch : 128 * cch + 128],
                            wv16[:],
                            start=True, stop=True, skip_group_check=True,
                        )
                    nc.vector.tensor_copy(
                        st_vsl[p][:, 1024 * half : 1024 * half + 1024], vps[:]
                    )

                def emit_slot(p, t):
                    bx16, qpt = st_bx[p], st_qpt[p]
                    KOFF = KOFF_P[p]
                    ext = 128 * (t + 1 + KOFF)
                    MW = 128 if p < 3 else 1152
                    mask = mD if p < 3 else mW
                    mlo = ext - MW  # mask col range [mlo, ext)
                    nt = (ext + 1023) // 1024
                    stiles = []
                    for i in range(nt):
                        w = min(1024, ext - 1024 * i)
                        st = spool.tile([128, 1024], f32, tag="s", name=f"st{p}_{t}_{i}{su}")
                        stiles.append((st, w))
                        for h in range(0, w, 512):
                            hw = min(512, w - h)
                            c0 = 1024 * i + h
                            overlaps_mask = c0 + hw > mlo
                            nc.tensor.matmul(
                                st[:, h : h + hw],
                                qpt[:, 128 * t : 128 * t + 128],
                                bx16[:, c0 : c0 + hw],
                                start=True, stop=not overlaps_mask,
                                skip_group_check=True,
                            )
                    # additive mask over [mlo, ext), in <=512 chunks aligned to
                    # the 512 grid so each lands in one score-matmul region
                    c = mlo
                    while c < ext:
                        hw = min(512 - (c % 512), ext - c)
                        i = c // 1024
                        st, _ = stiles[i]
                        nc.tensor.matmul(
                            st[:, c - 1024 * i : c - 1024 * i + hw],
                            ident16[:],
                            mask[:, c - mlo : c - mlo + hw],
                            start=False, stop=True, skip_group_check=True,
                        )
                        c += hw

                    Et = Etp.tile([128, S], f16, tag="Et", name=f"Et{p}{t}{su}")
                    for i, (st, w) in enumerate(stiles):
                        nc.scalar.activation(
                            Et[:, 1024 * i : 1024 * i + w], st[:, 0:w],
                            mybir.ActivationFunctionType.Exp,
                            bias=0.0, scale=1.0,
                        )

                    if "softmax" in skip:
                        return
                    # row max of Et (DVE), rec = 1/maxE, then scale Et in
                    # place (per-partition q ✓) with row-sum accum -> D~.
                    # The shift keeps EV/D~ consistent with the reference's
                    # max-shifted mixing weights; scatter needs no rescale.
                    nc.vector.tensor_scalar(
                        Et[:, 0:ext], Et[:, 0:ext], 1.0, None,
                        op0=mybir.AluOpType.mult, op1=mybir.AluOpType.max,
                        accum_out=st_max[p][:, t : t + 1],
                    )
                    nc.vector.reciprocal(
                        st_rec[p][:, t : t + 1], st_max[p][:, t : t + 1]
                    )
                    nc.vector.tensor_scalar(
                        Et[:, 0:ext], Et[:, 0:ext], st_rec[p][:, t : t + 1], None,
                        op0=mybir.AluOpType.mult, op1=mybir.AluOpType.add,
                        accum_out=st_du[p][:, t : t + 1],
                    )
                    if "transp" not in skip:
                        g = t // 4
                        key = (p, g)
                        if key not in st_etg:
                            st_etg[key] = etgp.tile(
                                [128, 16, 512], f16, tag="ETG", name=f"ETG{p}{g}{su}"
                            )
                        ch = ext // 128
                        nc.sync.dma_start_transpose(
                            st_etg[key][:, 0:ch, 128 * (t % 4) : 128 * (t % 4) + 128],
                            Et[:, 0:ext],
                        )

                def emit_rd(p, g):
                    """Per 4-slot group: scatter the group's D~ columns into
                    DenT. Off the critical path (only the epilogue reads
                    DenT): PE transpose -> DVE copy -> SWDGE flatten to a
                    partition-0 row -> gpsimd strided scatter."""
                    if "softmax" in skip or "adds" in skip:
                        return
                    rdT = evqp.tile([4, 128], f32, tag="ev", name=f"rdT{p}{g}{su}")
                    nc.tensor.transpose(
                        rdT[:], st_du[p][:, 4 * g : 4 * g + 4], ident[:]
                    )
                    rdT_sb = smp.tile([4, 128], f32, tag="rdsb", name=f"rdsb{p}{g}{su}")
                    nc.vector.tensor_copy(rdT_sb[:], rdT[:])
                    denrow = smp.tile([1, 512], f32, tag="drow", name=f"drow{p}{g}{su}")
                    nc.gpsimd.dma_start(denrow[:], rdT_sb[:])
                    base, stride = BASE_P[p], STRIDE_P[p]
                    denv = DenT[0:1, base::stride][:, 512 * g : 512 * g + 512]
                    if p < 2:
                        nc.gpsimd.tensor_copy(denv, denrow[:])
                    else:
                        nc.gpsimd.tensor_tensor(
                            denv, denv, denrow[:], op=mybir.AluOpType.add
                        )

                st_ev = {}

                def emit_ev_chunks(p, qh, s):
                    """Batch s (0..3) of EV quarter qh of piece p: ~1/4 of the
                    chunk-outer causal matmuls, interleaved between slots so PE
                    never runs a long EV burst that starves ACT of scores."""
                    if "ev" in skip or "softmax" in skip or "transp" in skip:
                        return
                    vsl = st_vsl[p]
                    KOFF = KOFF_P[p]
                    ch_hi = 4 * (qh + 1) + KOFF
                    if s == 0:
                        st_ev[(p, qh)] = evqp.tile(
                            [128, 512], f32, tag="ev", name=f"ev{p}{qh}{su}"
                        )
                    ev = st_ev[(p, qh)]
                    etg = st_etg[(p, qh)]
                    for cch in range(ch_hi * s // 4, ch_hi * (s + 1) // 4):
                        qs = max(0, 128 * (cch - KOFF) - 512 * qh)
                        nc.tensor.matmul(
                            ev[:, qs:512],
                            vsl[:, 128 * cch : 128 * cch + 128],
                            etg[:, cch, qs:512],
                            start=(cch == 0), stop=(cch == ch_hi - 1),
                            skip_group_check=True,
                        )
                    if s < 3 or "adds" in skip:
                        return
                    base, stride = BASE_P[p], STRIDE_P[p]
                    numv = NumT[:, base::stride][:, 512 * qh : 512 * qh + 512]
                    if p < 2:
                        nc.vector.tensor_copy(numv, ev[:])
                    else:
                        nc.vector.tensor_tensor(
                            numv, numv, ev[:], op=mybir.AluOpType.add
                        )

                # ---------- epilogue (per 512-col out quarter) ----------
                oview = out_half.rearrange("(r b p) c -> p r b c", p=128, b=4)

                def emit_epi(r):
                    if "epi" in skip or "softmax" in skip or "adds" in skip:
                        return
                    recr = epi.tile([1, 512], f32, tag="recr", name=f"recr{r}{su}")
                    nc.vector.reciprocal(recr[:], DenT[0:1, 512 * r : 512 * r + 512])
                    rexE = epi.tile([128, 512], f32, tag="rexE", name=f"rexE{r}{su}")
                    nc.gpsimd.partition_broadcast(rexE[:], recr[:])
                    nsc16 = epi.tile([128, 512], f16, tag="nsc", name=f"nsc{r}{su}")
                    nc.vector.tensor_tensor(
                        nsc16[:], NumT[:, 512 * r : 512 * r + 512], rexE[:],
                        op=mybir.AluOpType.mult,
                    )
                    ntT = epi.tile([128, 4, 128], f16, tag="ntT", name=f"ntT{r}{su}")
                    tr_eng = nc.scalar if r % 2 == 0 else nc.sync
                    tr_eng.dma_start_transpose(ntT[:], nsc16[:])
                    nto = epi.tile([128, 4, 128], f32, tag="nto", name=f"nto{r}{su}")
                    nc.vector.tensor_copy(nto[:], ntT[:])
                    eng = nc.sync if r % 2 == 0 else nc.scalar
                    eng.dma_start(oview[:, r], nto[:])

                # ---------- software pipeline (global slot scheduler) ----------
                G = [0, 16, 32, 48]
                TOT = 56
                sched = {g: [] for g in range(TOT)}
                tail = []

                def at(g, fn, *a):
                    if g < TOT:
                        sched[g].append((fn, a))
                    else:
                        # keep the slot index: tail events MUST run in slot
                        # order, else piece3's EV accumulation groups emit out
                        # of order and the evqp pool rotation hands a PSUM
                        # buffer to ev(3,0) before ev(3,1)'s scatter is
                        # emitted (deterministic clobber, rel err ~0.1)
                        tail.append((g, fn, a))

                # Slots run in DESCENDING ext order: the piece-end drain covers
                # the tiny slots while the next piece's big slots refill the
                # pipeline (and qp/V blocks overlap the small-slot drain).
                for p in range(NPIECE):
                    NS = NS_P[p]
                    if p > 0:
                        at(G[p] - 12, emit_dma, p)
                        at(G[p] - qpoff, emit_qp_block, p, 0)
                        if NS > 8:
                            at(G[p] - qpoff + 1, emit_qp_block, p, 1)
                        at(G[p] - qpoff + 2, emit_v_half, p, 0)
                        at(G[p] - qpoff + 3, emit_v_half, p, 1)
                    for qh in range(NS // 4):
                        # group qh's last slot (t=4qh) runs at tau = NS-1-4qh
                        at(G[p] + NS - 4 * qh, emit_rd, p, qh)
                        lead = lead3 if p < 3 else lead3 - 1
                        for s in range(4):
                            at(G[p] + NS - 1 - 4 * qh + lead + s,
                               emit_ev_chunks, p, qh, s)

                # out quarters r>=4 depend only on piece3's FIRST EV group
                # (qh=1, scatter done by slot 55) + early den groups of the
                # other pieces -> run them during piece3's qh=0 slots/tail.
                tail_evt = []
                emit_dma(0)
                emit_qp_block(0, 0)
                emit_qp_block(0, 1)
                emit_v_half(0, 0)
                emit_v_half(0, 1)
                for p in range(NPIECE):
                    for tau in range(NS_P[p]):
                        emit_slot(p, NS_P[p] - 1 - tau)
                        for fn, a in sched[G[p] + tau]:
                            fn(*a)
                # tail: piece3 qh0 EV + rd, interleaved with epilogue r=7..4
                # (ready), then r=3..0 (need qh0's scatter).
                tail.sort(key=lambda e: e[0])
                epi_early = [(emit_epi, (r,)) for r in (7, 6, 5, 4)]
                merged = []
                for i, (_, fn, a) in enumerate(tail):
                    merged.append((fn, a))
                    if i < len(epi_early):
                        merged.append(epi_early[i])
                merged += epi_early[len(tail):]
                for fn, a in merged:
                    fn(*a)
                for r in (3, 2, 1, 0):
                    emit_epi(r)

            if unroll_k:
                for _u in range(unroll_k):
                    _one_iter(f"_u{_u}")
            elif loop_k:
                with tc.For_i(0, loop_k, 1):
                    _one_iter("")
            else:
                _one_iter("")

    nc.finalize()
    return nc


# ---------------- host side ----------------

_SEG_POS = None


def _seg_positions():
    global _SEG_POS
    if _SEG_POS is None:
        segs = []
        for w, r in zip([2048, 4096, 8192], [1, 2, 4]):
            off = 1 % r
            for start in range(0, N, w):
                segs.append(np.arange(start, start + w)[off::r])
        _SEG_POS = segs  # 7 arrays of 2048
    return _SEG_POS


def _make_masks():
    q = np.arange(128)[:, None]
    k = np.arange(128)[None, :]
    tri = np.where(k <= q, 0.0, NEG).astype(np.float16)
    maskD = tri  # (128,128)
    maskW0 = np.concatenate(
        [tri, np.full((128, 1024), NEG, np.float16)], axis=1
    )  # rho=0: diag first, then window of masked future chunks
    maskW1 = np.concatenate(
        [np.zeros((128, 1024), np.float16), tri], axis=1
    )  # rho=1: 8 valid chunks then diag
    return maskD, maskW0, maskW1


_NC = None


def _get_nc():
    global _NC
    if _NC is None:
        _NC = build_nc()
    return _NC


def kernel(x, Wq, Wk, Wv, indices):
    x = np.asarray(x, dtype=np.float32)
    Wq = np.asarray(Wq, dtype=np.float32)
    Wk = np.asarray(Wk, dtype=np.float32)
    Wv = np.asarray(Wv, dtype=np.float32)

    M16 = (
        Wq.astype(np.float64) @ Wk.T.astype(np.float64) / math.sqrt(C)
    ).astype(np.float32).astype(np.float16)
    Wv16 = Wv.astype(np.float16)
    x16 = x.astype(np.float16)
    maskD, maskW0, maskW1 = _make_masks()
    segs = _seg_positions()

    in_maps = []
    for core in range(8):
        b = core // 2
        rho = core % 2
        piece_segs = [0, 1, 4, 6] if rho == 0 else [2, 3, 5, 6]
        bxT4 = np.empty((NPIECE, C, S), np.float16)
        for pi, si in enumerate(piece_segs):
            bxT4[pi] = x16[b][segs[si]].T
        qx3 = np.ascontiguousarray(
            bxT4[3][:, 0:1024] if rho == 0 else bxT4[3][:, 1024:2048]
        )
        in_maps.append(
            {
                "bxT4": bxT4,
                "qx3": qx3,
                "maskD": maskD,
                "maskW": maskW0 if rho == 0 else maskW1,
                "Mt": M16,
                "Wvt": Wv16,
            }
        )

    nc = _get_nc()
    res = run_bass_kernel_spmd(nc, in_maps, list(range(8))).results

    out = np.empty((B, N, C), np.float32)
    for b in range(B):
        out[b, :HALF] = res[2 * b]["out_half"]
        out[b, HALF:] = res[2 * b + 1]["out_half"]
    return out


def kernel_profiled(x, Wq, Wk, Wv, indices, **trace_kwargs):
    """Like kernel() but returns (out, BassKernelResults) with trace enabled."""
    global run_bass_kernel_spmd
    orig = run_bass_kernel_spmd
    holder = {}

    def wrapper(nc, in_maps, core_ids, **kw):
        r = orig(nc, in_maps, core_ids, trace=True, **trace_kwargs)
        holder["r"] = r
        return r

    run_bass_kernel_spmd = wrapper
    try:
        out = kernel(x, Wq, Wk, Wv, indices)
    finally:
        run_bass_kernel_spmd = orig
    return out, holder["r"]
